# revision 1
# baseline (speedup 1.0000x reference)
"""ARMA GNN (K=3, T=2, two convs) on 8 TRN2 NeuronCores.

Strategy: nodes sharded 3750/core; edges sharded by dst core, sorted by dst
window (128 nodes); propagate = dma_gather of source rows from a replicated
bf16 table + one-hot scatter matmuls on TensorE into PSUM per dst window.
gcn_norm is folded as propagate(h) = D @ A @ D @ h with D=diag(deg^-1/2):
gather tables are pre-scaled by D (rows), scatter outputs scaled by D
(per-partition multiply). Dense per-stack matmuls run feature-major
([feat, nodes]) and are node-parallel; tables for the next propagate are
built via PE transposes. Cross-core replication of propagate tables uses
ncfw AllGather (bf16).
"""
import os
import numpy as np
import ml_dtypes

import concourse.bass as bass
import concourse.mybir as mybir
import concourse.tile as tile
import concourse.bacc as bacc
from concourse.masks import make_identity
from concourse.bass_utils import run_bass_kernel_spmd

NCORE = 8
N = 30000
EDG = 300000
NL = N // NCORE          # 3750 nodes per core
K = 3
F_IN = 256
F_HID = 256
F_OUT = 64
NWIN = (NL + 127) // 128  # 30 windows (last has 38 nodes)
P = 128
BF = mybir.dt.bfloat16
F32 = mybir.dt.float32
I16 = mybir.dt.int16

LAST_EXEC_NS = None


def _t1_layout():
    if NL < 3 * P:
        return [NL]
    a = ((NL // 3 + P - 1) // P) * P
    return [a, a, NL - 2 * a]


def _wrap16(idx):
    """int16 idx array -> [128, len/16] gather layout (16-part wrap, 8x tile)."""
    w = idx.reshape(-1, 16).T.astype(np.int16).copy()
    return np.tile(w, (8, 1))


def _prep(x, edge_index, init_w1, w1, root_w1, bias1, init_w2, w2, root_w2, bias2):
    """Host-side sharding/staging. Returns (in_maps, T_W, NT)."""
    x = np.asarray(x, np.float32)
    ei = np.asarray(edge_index, np.int64)
    row0, col0 = ei[0], ei[1]
    deg0 = np.bincount(col0, minlength=N)

    # degree-balanced node permutation: serpentine-deal nodes (sorted by
    # in-degree) over the 240 (core, window) bins so per-window edge maxima
    # shrink -> fewer padded scatter tiles (T_W).
    TAIL = NL - (NWIN - 1) * P                      # 38 nodes in last window
    bins = [(c, w) for w in range(NWIN - 1) for c in range(NCORE)]
    tails = [(c, NWIN - 1) for c in range(NCORE)]
    cap = {b: P for b in bins}
    for b in tails:
        cap[b] = TAIL
    order_nodes = np.argsort(-deg0, kind="stable")
    allbins = bins + tails
    assign = {b: [] for b in allbins}
    r = 0
    ptr0 = 0
    while ptr0 < N:
        seq = allbins if (r % 2 == 0) else allbins[::-1]
        for b in seq:
            if ptr0 >= N:
                break
            if len(assign[b]) < cap[b]:
                assign[b].append(order_nodes[ptr0])
                ptr0 += 1
        r += 1
    perm = np.empty(N, np.int64)                    # perm[new] = old
    for (c, w), lst in assign.items():
        base = c * NL + w * P
        perm[base:base + len(lst)] = lst
    inv = np.empty(N, np.int64)
    inv[perm] = np.arange(N)

    row, col = inv[row0], inv[col0]
    x = x[perm]
    deg = np.bincount(col, minlength=N).astype(np.float32)
    dis = np.where(deg > 0, 1.0 / np.sqrt(np.maximum(deg, 1.0)), 0.0).astype(np.float32)

    xs = (x * dis[:, None]).astype(ml_dtypes.bfloat16)          # gather table p0
    x_bf = x.astype(ml_dtypes.bfloat16)

    # per-core edge lists grouped by dst window
    core_of = col // NL
    dloc = col - core_of * NL
    win = dloc // P
    rel = dloc - win * P
    # counts per (core, window)
    cnt = np.zeros((NCORE, NWIN), np.int64)
    np.add.at(cnt, (core_of, win), 1)
    T_W = [int(np.ceil(cnt[:, w].max() / P)) for w in range(NWIN)]
    T_W = [max(t, 1) for t in T_W]
    NT = sum(T_W)
    EP = NT * P

    src_pad = np.zeros((NCORE, EP), np.int64)
    rel_pad = np.full((NCORE, EP), -1, np.int64)
    order = np.lexsort((win, core_of))
    srcs = row[order]
    rels = rel[order]
    cores_s = core_of[order]
    wins_s = win[order]
    # start offsets per (core, window) in the sorted array
    ptr = 0
    woff = np.concatenate([[0], np.cumsum(np.array(T_W) * P)])[:-1]
    for c in range(NCORE):
        for w in range(NWIN):
            n = cnt[c, w]
            if n:
                sl = slice(ptr, ptr + n)
                assert (cores_s[sl] == c).all() and (wins_s[sl] == w).all()
                src_pad[c, woff[w]:woff[w] + n] = srcs[sl]
                rel_pad[c, woff[w]:woff[w] + n] = rels[sl]
                ptr += n
    assert ptr == EDG

    # weights (bf16, host-folded)
    iw1 = np.asarray(init_w1, np.float32)
    w1f = np.asarray(w1, np.float32)
    rw1 = np.asarray(root_w1, np.float32)
    b1 = np.asarray(bias1, np.float32).reshape(K, F_HID)
    iw2 = np.asarray(init_w2, np.float32)
    w2f = np.asarray(w2, np.float32)
    rw2 = np.asarray(root_w2, np.float32)
    b2 = np.asarray(bias2, np.float32).reshape(K, F_OUT)

    def pack_pair(wm):  # [K, 256, 64] -> planes [2, 256, 128]
        pl = np.zeros((2, F_HID, 128), np.float32)
        pl[0, :, :64] = wm[0]
        pl[0, :, 64:] = wm[1]
        pl[1, :, :64] = wm[2]
        return pl

    rw2p = pack_pair(rw2 / 3.0).astype(ml_dtypes.bfloat16)
    iw2p = pack_pair(iw2 / 3.0).astype(ml_dtypes.bfloat16)
    w23 = np.zeros((P, 2, 64), np.float32)   # partition-packed w2/3
    w23[0:64, 0] = w2f[0] / 3.0
    w23[64:128, 0] = w2f[1] / 3.0
    w23[0:64, 1] = w2f[2] / 3.0
    wrm = (rw2.sum(axis=0) / 9.0).astype(ml_dtypes.bfloat16)   # y' = 3y
    b2p = np.zeros((P, 2), np.float32)
    b2p[0:64, 0] = b2[0]
    b2p[64:128, 0] = b2[1]
    b2p[0:64, 1] = b2[2]
    brm = np.zeros((P, 1), np.float32)
    brm[0:64, 0] = b2.sum(axis=0) / 3.0
    b1sb = np.zeros((P, K * 2), np.float32)
    for k in range(K):
        for fo in range(2):
            b1sb[:, k * 2 + fo] = b1[k, fo * P:(fo + 1) * P]

    common = {
        "xs": xs,
        "rw1": rw1.astype(ml_dtypes.bfloat16),
        "iw1": iw1.astype(ml_dtypes.bfloat16),
        "w1": w1f.astype(ml_dtypes.bfloat16),
        "rw2p": rw2p, "iw2p": iw2p,
        "w23": w23.astype(ml_dtypes.bfloat16),
        "wrm": wrm, "b2p": b2p, "brm": brm, "b1": b1sb,
    }
    in_maps = []
    for c in range(NCORE):
        sl = slice(c * NL, (c + 1) * NL)
        disw = np.zeros((P, NWIN), np.float32)
        for w in range(NWIN):
            n = min(P, NL - w * P)
            disw[:n, w] = dis[c * NL + w * P: c * NL + w * P + n]
        m = dict(common)
        m["xT"] = np.ascontiguousarray(x_bf[sl].T)              # [256, 3750]
        m["srcidx"] = _wrap16(src_pad[c])
        r1 = src_pad[c] % NL
        c1 = src_pad[c] // NL
        CHS = np.array(_t1_layout())
        cums = np.concatenate([[0], np.cumsum(CHS)])
        j1 = np.searchsorted(cums, r1, side="right") - 1
        m["srcidx1"] = _wrap16(NCORE * cums[j1] + c1 * CHS[j1] + (r1 - cums[j1]))
        m["dstrel"] = rel_pad[c].reshape(NT, P).T.astype(np.int16).copy()
        m["disw"] = disw
        in_maps.append(m)
    return in_maps, T_W, NT, perm


def _build(T_W, NT):
    NTT = sum(T_W)
    assert NTT == NT
    EP = NT * P
    nc = bacc.Bacc("TRN2", target_bir_lowering=False, debug=False, num_devices=NCORE)
    dp = nc.declare_dram_parameter
    xs_t = dp("xs", [N, F_IN], BF, isOutput=False)
    xT_t = dp("xT", [F_IN, NL], BF, isOutput=False)
    src_t = dp("srcidx", [P, EP // 16], I16, isOutput=False)
    src1_t = dp("srcidx1", [P, EP // 16], I16, isOutput=False)
    rel_t = dp("dstrel", [P, NT], I16, isOutput=False)
    disw_t = dp("disw", [P, NWIN], F32, isOutput=False)
    rw1_t = dp("rw1", [K, F_IN, F_HID], BF, isOutput=False)
    iw1_t = dp("iw1", [K, F_HID, F_HID], BF, isOutput=False)
    w1_t = dp("w1", [K, F_HID, F_HID], BF, isOutput=False)
    rw2p_t = dp("rw2p", [2, F_HID, P], BF, isOutput=False)
    iw2p_t = dp("iw2p", [2, F_HID, P], BF, isOutput=False)
    w23_t = dp("w23", [P, 2, 64], BF, isOutput=False)
    wrm_t = dp("wrm", [F_HID, F_OUT], BF, isOutput=False)
    b1_t = dp("b1", [P, K * 2], F32, isOutput=False)
    b2p_t = dp("b2p", [P, 2], F32, isOutput=False)
    brm_t = dp("brm", [P, 1], F32, isOutput=False)
    out_t = dp("out", [NL, F_OUT], F32, isOutput=True)

    CHS = _t1_layout()
    CUMS = np.concatenate([[0], np.cumsum(CHS)]).astype(int)
    t1_locs = [nc.dram_tensor(f"t1_loc{j}", [sz, 768], BF) for j, sz in enumerate(CHS)]
    t2_loc = nc.dram_tensor("t2_loc", [NL, 256], BF)
    t3_loc = nc.dram_tensor("t3_loc", [NL, 128], BF)
    t1_full = nc.dram_tensor("t1_full", [N, 768], BF, addr_space="Shared")
    t2_full = nc.dram_tensor("t2_full", [N, 256], BF, addr_space="Shared")
    t3_full = nc.dram_tensor("t3_full", [N, 128], BF, addr_space="Shared")

    woff_t = np.concatenate([[0], np.cumsum(T_W)])  # tile offsets per window
    # gather chunks of <=8 tiles over the global tile sequence
    chunks = []
    lo = 0
    while lo < NT:
        hi = min(lo + 8, NT)
        chunks.append((lo, hi))
        lo = hi
    chunk_of = {}
    for ci, (lo, hi) in enumerate(chunks):
        for t in range(lo, hi):
            chunk_of[t] = (ci, t - lo)

    NCH = [c for c in range(0, NL, 512)]  # dense node chunk starts

    with tile.TileContext(nc) as tc:
        with (
            tc.tile_pool(name="const", bufs=1) as cp,
            tc.tile_pool(name="resid", bufs=1) as rp,
            
            tc.tile_pool(name="ohp", bufs=12) as ohp,
            tc.tile_pool(name="wk", bufs=4) as wk,
            tc.tile_pool(name="tabp", bufs=5) as tabp,
            
            tc.tile_pool(name="psT", bufs=2, space="PSUM") as psT,
            tc.tile_pool(name="psD", bufs=2, space="PSUM") as psD,
        ):
            # ---- constants ----
            iota = cp.tile([P, P], F32)
            nc.gpsimd.iota(iota[:], pattern=[[1, P]], base=0, channel_multiplier=0,
                           allow_small_or_imprecise_dtypes=True)
            ident = cp.tile([P, P], BF)
            make_identity(nc, ident[:])
            src16 = cp.tile([P, EP // 16], I16)
            nc.sync.dma_start(out=src16[:], in_=src_t[:])
            src16b = cp.tile([P, EP // 16], I16)
            nc.sync.dma_start(out=src16b[:], in_=src1_t[:])
            rel16 = cp.tile([P, NT], I16)
            nc.sync.dma_start(out=rel16[:], in_=rel_t[:])
            relf = cp.tile([P, NT], F32)
            nc.vector.tensor_copy(out=relf[:], in_=rel16[:])
            disw = cp.tile([P, NWIN], F32)
            nc.sync.dma_start(out=disw[:], in_=disw_t[:])
            b1c = cp.tile([P, K * 2], F32)
            nc.sync.dma_start(out=b1c[:], in_=b1_t[:])
            b2c = cp.tile([P, 2], F32)
            nc.sync.dma_start(out=b2c[:], in_=b2p_t[:])
            brmc = cp.tile([P, 1], F32)
            nc.sync.dma_start(out=brmc[:], in_=brm_t[:])
            # weights: [128, k*2+fi, fout-cols]
            rw1s = cp.tile([P, K * 2, F_HID], BF)
            iw1s = cp.tile([P, K * 2, F_HID], BF)
            w1s = cp.tile([P, K * 2, F_HID], BF)
            for k in range(K):
                for fi in range(2):
                    nc.sync.dma_start(out=rw1s[:, k * 2 + fi, :], in_=rw1_t[k, fi * P:(fi + 1) * P, :])
                    nc.sync.dma_start(out=iw1s[:, k * 2 + fi, :], in_=iw1_t[k, fi * P:(fi + 1) * P, :])
                    nc.sync.dma_start(out=w1s[:, k * 2 + fi, :], in_=w1_t[k, fi * P:(fi + 1) * P, :])
            rw2s = cp.tile([P, 2 * 2, P], BF)
            iw2s = cp.tile([P, 2 * 2, P], BF)
            for pl in range(2):
                for fi in range(2):
                    nc.sync.dma_start(out=rw2s[:, pl * 2 + fi, :], in_=rw2p_t[pl, fi * P:(fi + 1) * P, :])
                    nc.sync.dma_start(out=iw2s[:, pl * 2 + fi, :], in_=iw2p_t[pl, fi * P:(fi + 1) * P, :])
            w23s = cp.tile([P, 2, 64], BF)
            nc.sync.dma_start(out=w23s[:], in_=w23_t[:])
            wrms = cp.tile([P, 2, F_OUT], BF)
            nc.sync.dma_start(out=wrms[:], in_=wrm_t[:].rearrange("(a p) f -> p a f", p=P))
            xT = rp.tile([P, 2, NL], BF, tag="xTt")
            nc.sync.dma_start(out=xT[:], in_=xT_t[:].rearrange("(a p) n -> p a n", p=P))

            # ---- residents ----
            CW = [min(512, NL - b) for b in range(0, NL, 512)]
            root1T = [rp.tile([P, K * 2, CW[j]], BF, tag=f"slC{j}", name=f"root1T{j}") for j in range(len(CW))]
            SxT = [rp.tile([P, 2, CW[j]], BF, tag=f"slB{j}", name=f"SxT{j}") for j in range(len(CW))]
            ypT = [rp.tile([P, 2, CW[j]], BF, tag=f"slA{j}", name=f"ypT{j}") for j in range(len(CW))]
            root2T = [rp.tile([P, 2, CW[j]], BF, tag=f"slB{j}", name=f"root2T{j}") for j in range(len(CW))]
            h3T = [rp.tile([P, 2, CW[j]], BF, tag=f"slC{j}", name=f"h3T{j}") for j in range(len(CW))]
            rmnm = rp.tile([P, NWIN, F_OUT], BF)  # mean_k root2, node-major

            GO = int(os.environ.get("GO", "99"))
            # ---- stage B: root1T = x@rw1 + b1 (feature-major) ----
            if GO >= 1:
                for k in range(K):
                    for fo in range(2):
                        for nb in NCH:
                            nn_ = min(512, NL - nb)
                            ps = psD.tile([P, 512], F32, tag="dense")
                            for fi in range(2):
                                nc.tensor.matmul(
                                    out=ps[:, :nn_],
                                    lhsT=rw1s[:, k * 2 + fi, fo * P:(fo + 1) * P],
                                    rhs=xT[:, fi, nb:nb + nn_],
                                    start=(fi == 0), stop=(fi == 1))
                            nc.vector.tensor_scalar(
                                out=root1T[nb // 512][:, k * 2 + fo, :nn_], in0=ps[:, :nn_],
                                scalar1=b1c[:, k * 2 + fo:k * 2 + fo + 1], scalar2=None,
                                op0=mybir.AluOpType.add)

            # ---- generic propagate ----
            NPROP = int(os.environ.get("NPROP", "99"))
            NAG = int(os.environ.get("NAG", "99"))
            prop_ctr = [0]

            def propagate(table, W, epilogue, idxs=None):
                """scatter-accumulate per window; epilogue(w, psum_acc)."""
                my_idx = prop_ctr[0]
                prop_ctr[0] += 1
                pa = tc.tile_pool(name=f"psA_{table.name}", bufs=(2 if W > 256 else 4), space="PSUM")
                psAx = pa.__enter__()
                if my_idx >= NPROP:
                    for w in range(NWIN):
                        acc = psAx.tile([P, W if W > 256 else 256], F32, tag="acc", name=f"acc_{my_idx}_{w}")
                        nc.vector.memset(acc[:], 0)
                        epilogue(w, acc)
                    pa.__exit__(None, None, None)
                    return
                mp = tc.tile_pool(name=f"msg_{table.name}", bufs=3)
                mpool = mp.__enter__()
                msgs = {}
                for ci, (lo, hi) in enumerate(chunks):
                    m = mpool.tile([P, 8, W], BF, tag="msg", name=f"msg_{table.name}_{ci}")
                    ni = (hi - lo) * P
                    nc.gpsimd.dma_gather(
                        out_ap=m[:, :hi - lo, :], in_ap=table[:],
                        idxs_ap=(idxs if idxs is not None else src16)[:, lo * 8:lo * 8 + ni // 16],
                        num_idxs=ni, num_idxs_reg=ni, elem_size=W)
                    msgs[ci] = m
                for w in range(NWIN):
                    acc = psAx.tile([P, W if W > 256 else 256], F32, tag="acc", name=f"accr_{my_idx}_{w}")
                    for ti in range(T_W[w]):
                        gt = int(woff_t[w]) + ti
                        ci, slot = chunk_of[gt]
                        oh = ohp.tile([P, P], BF, tag="oh")
                        nc.vector.tensor_scalar(
                            out=oh[:], in0=iota[:],
                            scalar1=relf[:, gt:gt + 1], scalar2=None,
                            op0=mybir.AluOpType.is_equal)
                        st = (ti == 0)
                        sp = (ti == T_W[w] - 1)
                        if W > 512:
                            for c0, c1 in ((0, 512), (512, W)):
                                nc.tensor.matmul(out=acc[:, c0:c1], lhsT=oh[:],
                                                 rhs=msgs[ci][:, slot, c0:c1],
                                                 start=st, stop=sp)
                        else:
                            nc.tensor.matmul(out=acc[:, :W], lhsT=oh[:],
                                             rhs=msgs[ci][:, slot, :],
                                             start=st, stop=sp)
                    epilogue(w, acc)
                mp.__exit__(None, None, None)
                pa.__exit__(None, None, None)

            # ---- propagate 0: Sx from xs ----
            if GO >= 2:
                def ep0(w, acc):
                    nn_ = min(P, NL - w * P)
                    snm = wk.tile([P, F_IN], BF, tag="snm")
                    nc.vector.tensor_scalar(out=snm[:], in0=acc[:, :F_IN],
                                            scalar1=disw[:, w:w + 1], scalar2=None,
                                            op0=mybir.AluOpType.mult)
                    for fc in range(2):
                        pt = psT.tile([P, P], BF, tag="tr")
                        nc.tensor.transpose(out=pt[:], in_=snm[:, fc * P:(fc + 1) * P], identity=ident[:])
                        nc.vector.tensor_copy(out=SxT[(w * P) // 512][:, fc, (w * P) % 512:(w * P) % 512 + nn_], in_=pt[:, :nn_])
                propagate(xs_t, F_IN, ep0)

            # ---- stage D: h1 -> g1 -> t1_loc ----
            if GO >= 3:
                for nb in NCH:
                    nn_ = min(512, NL - nb)
                    nsub = [s for s in range(0, nn_, P)]
                    tabs = [tabp.tile([P, 768], BF, tag="tab1", name=f"tab1_{nb}_{i}") for i, _ in enumerate(nsub)]
                    for k in range(K):
                        h1 = [wk.tile([P, 512], BF, tag="h1", name=f"h1_{nb}_{k}_{i}") for i in range(2)]
                        for fo in range(2):
                            ps = psD.tile([P, 512], F32, tag="dense")
                            for fi in range(2):
                                nc.tensor.matmul(out=ps[:, :nn_],
                                                 lhsT=iw1s[:, k * 2 + fi, fo * P:(fo + 1) * P],
                                                 rhs=SxT[nb // 512][:, fi, :nn_],
                                                 start=(fi == 0), stop=False)
                            nc.tensor.matmul(out=ps[:, :nn_], lhsT=ident[:],
                                             rhs=root1T[nb // 512][:, k * 2 + fo, :nn_],
                                             start=False, stop=True)
                            nc.scalar.activation(out=h1[fo][:, :nn_], in_=ps[:, :nn_],
                                                 func=mybir.ActivationFunctionType.Relu)
                        for fo in range(2):
                            ps = psD.tile([P, 512], F32, tag="dense")
                            for fi in range(2):
                                nc.tensor.matmul(out=ps[:, :nn_],
                                                 lhsT=w1s[:, k * 2 + fi, fo * P:(fo + 1) * P],
                                                 rhs=h1[fi][:, :nn_],
                                                 start=(fi == 0), stop=(fi == 1))
                            gb = wk.tile([P, 512], BF, tag="gb")
                            nc.vector.tensor_copy(out=gb[:, :nn_], in_=ps[:, :nn_])
                            for si, s in enumerate(nsub):
                                pt = psT.tile([P, P], BF, tag="tr")
                                nc.tensor.transpose(out=pt[:], in_=gb[:, s:s + P], identity=ident[:])
                                w0 = (nb + s) // P
                                nc.vector.tensor_scalar(
                                    out=tabs[si][:, k * 256 + fo * P:k * 256 + (fo + 1) * P],
                                    in0=pt[:], scalar1=disw[:, w0:w0 + 1], scalar2=None,
                                    op0=mybir.AluOpType.mult)
                    for si, s in enumerate(nsub):
                        r = nb + s
                        rows = min(P, NL - r)
                        j = int(np.searchsorted(CUMS, r, side="right")) - 1
                        nc.sync.dma_start(out=t1_locs[j][r - CUMS[j]:r - CUMS[j] + rows, :],
                                          in_=tabs[si][:rows, :])

                bnds = [(NCORE * CUMS[j], NCORE * CUMS[j + 1]) for j in range(len(CHS))]
                if NAG > prop_ctr[0] - 1:
                    for j in range(len(CHS)):
                        nc.gpsimd.collective_compute(
                            "AllGather", mybir.AluOpType.bypass,
                            replica_groups=[list(range(NCORE))],
                            ins=[t1_locs[j][:]], outs=[t1_full[bnds[j][0]:bnds[j][1], :]])
                else:
                    for j in range(len(CHS)):
                        nc.sync.dma_start(out=t1_full[bnds[j][0]:bnds[j][0] + CHS[j], :], in_=t1_locs[j][:])

            # ---- propagate 1 + h2 + y' ----
            if GO >= 4:
                def ep1(w, acc):
                    nn_ = min(P, NL - w * P)
                    snm = wk.tile([P, 768], BF, tag="snm1")
                    nc.vector.tensor_scalar(out=snm[:], in0=acc[:],
                                            scalar1=disw[:, w:w + 1], scalar2=None,
                                            op0=mybir.AluOpType.mult)
                    for k in range(K):
                        for fo in range(2):
                            pt = psT.tile([P, P], BF, tag="tr")
                            nc.tensor.transpose(out=pt[:], in_=snm[:, k * 256 + fo * P:k * 256 + (fo + 1) * P],
                                                identity=ident[:])
                            hsum = wk.tile([P, P], BF, tag="hsum")
                            nc.vector.tensor_tensor(out=hsum[:, :nn_], in0=pt[:, :nn_],
                                                    in1=root1T[(w * P) // 512][:, k * 2 + fo, (w * P) % 512:(w * P) % 512 + nn_],
                                                    op=mybir.AluOpType.add)
                            if k == 0:
                                nc.scalar.activation(out=ypT[(w * P) // 512][:, fo, (w * P) % 512:(w * P) % 512 + nn_], in_=hsum[:, :nn_],
                                                     func=mybir.ActivationFunctionType.Relu)
                            else:
                                hr = wk.tile([P, P], BF, tag="hr")
                                nc.scalar.activation(out=hr[:, :nn_], in_=hsum[:, :nn_],
                                                     func=mybir.ActivationFunctionType.Relu)
                                nc.vector.tensor_tensor(out=ypT[(w * P) // 512][:, fo, (w * P) % 512:(w * P) % 512 + nn_],
                                                        in0=ypT[(w * P) // 512][:, fo, (w * P) % 512:(w * P) % 512 + nn_],
                                                        in1=hr[:, :nn_], op=mybir.AluOpType.add)
                propagate(t1_full, 768, ep1, idxs=src16b)

            # ---- stage G: root2T, rm, g2 -> t2_loc ----
            if GO >= 5:
                for nb in NCH:
                    nn_ = min(512, NL - nb)
                    nsub = [s for s in range(0, nn_, P)]
                    tabs = [tabp.tile([P, 256], BF, tag="tab2", name=f"tab2_{nb}_{i}") for i, _ in enumerate(nsub)]
                    for pl in range(2):
                        # root2T plane
                        ps = psD.tile([P, 512], F32, tag="dense")
                        for fi in range(2):
                            nc.tensor.matmul(out=ps[:, :nn_], lhsT=rw2s[:, pl * 2 + fi, :],
                                             rhs=ypT[nb // 512][:, fi, :nn_],
                                             start=(fi == 0), stop=(fi == 1))
                        nc.vector.tensor_scalar(out=root2T[nb // 512][:, pl, :nn_], in0=ps[:, :nn_],
                                                scalar1=b2c[:, pl:pl + 1], scalar2=None,
                                                op0=mybir.AluOpType.add)
                        # g2 plane
                        ps2 = psD.tile([P, 512], F32, tag="dense")
                        for fi in range(2):
                            nc.tensor.matmul(out=ps2[:, :nn_], lhsT=iw2s[:, pl * 2 + fi, :],
                                             rhs=ypT[nb // 512][:, fi, :nn_],
                                             start=(fi == 0), stop=(fi == 1))
                        gb = wk.tile([P, 512], BF, tag="gb")
                        nc.vector.tensor_copy(out=gb[:, :nn_], in_=ps2[:, :nn_])
                        for si, s in enumerate(nsub):
                            pt = psT.tile([P, P], BF, tag="tr")
                            nc.tensor.transpose(out=pt[:], in_=gb[:, s:s + P], identity=ident[:])
                            w0 = (nb + s) // P
                            if pl == 0:
                                nc.vector.tensor_scalar(out=tabs[si][:, 0:P], in0=pt[:],
                                                        scalar1=disw[:, w0:w0 + 1], scalar2=None,
                                                        op0=mybir.AluOpType.mult)
                            else:
                                nc.vector.tensor_scalar(out=tabs[si][:, P:P + 64], in0=pt[:, :64],
                                                        scalar1=disw[:, w0:w0 + 1], scalar2=None,
                                                        op0=mybir.AluOpType.mult)
                                nc.vector.memset(tabs[si][:, 192:256], 0)
                    # rm (mean_k root2) node-major
                    psr = psD.tile([P, 512], F32, tag="dense")
                    for fi in range(2):
                        nc.tensor.matmul(out=psr[:64, :nn_], lhsT=wrms[:, fi, :],
                                         rhs=ypT[nb // 512][:, fi, :nn_],
                                         start=(fi == 0), stop=(fi == 1))
                    rmf = wk.tile([P, 512], BF, tag="rmf")
                    nc.vector.tensor_scalar(out=rmf[:64, :nn_], in0=psr[:64, :nn_],
                                            scalar1=brmc[:64, 0:1], scalar2=None,
                                            op0=mybir.AluOpType.add)
                    for si, s in enumerate(nsub):
                        pt = psT.tile([P, P], BF, tag="tr")
                        nc.tensor.matmul(out=pt[:, :64], lhsT=rmf[:64, s:s + P],
                                         rhs=ident[:64, :64], is_transpose=True)
                        w0 = (nb + s) // P
                        nc.vector.tensor_copy(out=rmnm[:, w0, :], in_=pt[:, :64])
                    for si, s in enumerate(nsub):
                        rows = min(P, NL - (nb + s))
                        nc.sync.dma_start(out=t2_loc[nb + s:nb + s + rows, :], in_=tabs[si][:rows, :])

                if NAG > prop_ctr[0] - 1:
                    nc.gpsimd.collective_compute(
                        "AllGather", mybir.AluOpType.bypass,
                        replica_groups=[list(range(NCORE))],
                        ins=[t2_loc[:]], outs=[t2_full[:]])
                else:
                    nc.sync.dma_start(out=t2_full[0:NL, :], in_=t2_loc[:])

            # ---- propagate 2 + h3 ----
            if GO >= 6:
                def ep2(w, acc):
                    nn_ = min(P, NL - w * P)
                    snm = wk.tile([P, 256], BF, tag="snm2")
                    nc.vector.tensor_scalar(out=snm[:], in0=acc[:, :256],
                                            scalar1=disw[:, w:w + 1], scalar2=None,
                                            op0=mybir.AluOpType.mult)
                    for pl in range(2):
                        pt = psT.tile([P, P], BF, tag="tr")
                        nc.tensor.transpose(out=pt[:], in_=snm[:, pl * P:(pl + 1) * P], identity=ident[:])
                        nc.vector.tensor_tensor(out=h3T[(w * P) // 512][:, pl, (w * P) % 512:(w * P) % 512 + nn_], in0=pt[:, :nn_],
                                                in1=root2T[(w * P) // 512][:, pl, (w * P) % 512:(w * P) % 512 + nn_],
                                                op=mybir.AluOpType.add)
                propagate(t2_full, 256, ep2)

            # ---- stage: gm = mean_k g3 -> t3_loc ----
            if GO >= 7:
                for nb in NCH:
                    nn_ = min(512, NL - nb)
                    nsub = [s for s in range(0, nn_, P)]
                    ps = psD.tile([P, 512], F32, tag="dense")
                    nc.tensor.matmul(out=ps[:64, :nn_], lhsT=w23s[:, 0, :],
                                     rhs=h3T[nb // 512][:, 0, :nn_],
                                     start=True, stop=False)
                    nc.tensor.matmul(out=ps[:64, :nn_], lhsT=w23s[0:64, 1, :],
                                     rhs=h3T[nb // 512][0:64, 1, :nn_],
                                     start=False, stop=True)
                    gmf = wk.tile([P, 512], BF, tag="gmf")
                    nc.vector.tensor_copy(out=gmf[:64, :nn_], in_=ps[:64, :nn_])
                    for si, s in enumerate(nsub):
                        pt = psT.tile([P, P], BF, tag="tr")
                        nc.tensor.matmul(out=pt[:, :64], lhsT=gmf[:64, s:s + P],
                                         rhs=ident[:64, :64], is_transpose=True)
                        w0 = (nb + s) // P
                        tb = tabp.tile([P, 128], BF, tag="tab3")
                        nc.vector.tensor_scalar(out=tb[:, :64], in0=pt[:, :64],
                                                scalar1=disw[:, w0:w0 + 1], scalar2=None,
                                                op0=mybir.AluOpType.mult)
                        nc.vector.memset(tb[:, 64:], 0)
                        rows = min(P, NL - (nb + s))
                        nc.sync.dma_start(out=t3_loc[nb + s:nb + s + rows, :], in_=tb[:rows, :])

                if NAG > prop_ctr[0] - 1:
                    nc.gpsimd.collective_compute(
                        "AllGather", mybir.AluOpType.bypass,
                        replica_groups=[list(range(NCORE))],
                        ins=[t3_loc[:]], outs=[t3_full[:]])
                else:
                    nc.sync.dma_start(out=t3_full[0:NL, :], in_=t3_loc[:])

            # ---- propagate 3 + logsoftmax -> out ----
            if GO >= 8:
                def ep3(w, acc):
                    rows = min(P, NL - w * P)
                    z = wk.tile([P, 64], F32, tag="z")
                    nc.vector.tensor_scalar(out=z[:], in0=acc[:, :64],
                                            scalar1=disw[:, w:w + 1], scalar2=None,
                                            op0=mybir.AluOpType.mult)
                    nc.vector.tensor_tensor(out=z[:], in0=z[:], in1=rmnm[:, w, :],
                                            op=mybir.AluOpType.add)
                    if int(os.environ.get("SKIP_SOFTMAX", "0")):
                        nc.sync.dma_start(out=out_t[w * P:w * P + rows, :], in_=z[:rows, :])
                        return
                    mx = wk.tile([P, 1], F32, tag="mx")
                    nc.vector.tensor_reduce(out=mx[:], in_=z[:], axis=mybir.AxisListType.X,
                                            op=mybir.AluOpType.max)
                    nmx = wk.tile([P, 1], F32, tag="nmx")
                    nc.vector.tensor_scalar(out=nmx[:], in0=mx[:], scalar1=-1.0, scalar2=None,
                                            op0=mybir.AluOpType.mult)
                    ex = wk.tile([P, 64], F32, tag="ex")
                    sume = wk.tile([P, 1], F32, tag="sume")
                    nc.scalar.activation(out=ex[:], in_=z[:],
                                         func=mybir.ActivationFunctionType.Exp,
                                         bias=nmx[:, 0:1], scale=1.0, accum_out=sume[:, 0:1])
                    lse = wk.tile([P, 1], F32, tag="lse")
                    nc.scalar.activation(out=lse[:], in_=sume[:],
                                         func=mybir.ActivationFunctionType.Ln)
                    o = wk.tile([P, 64], F32, tag="o")
                    nc.vector.tensor_scalar(out=o[:], in0=z[:], scalar1=mx[:, 0:1],
                                            scalar2=lse[:, 0:1],
                                            op0=mybir.AluOpType.subtract,
                                            op1=mybir.AluOpType.subtract)
                    nc.sync.dma_start(out=out_t[w * P:w * P + rows, :], in_=o[:rows, :])
                propagate(t3_full, 128, ep3)

    nc.compile()
    return nc


def kernel(**inputs):
    global LAST_EXEC_NS
    in_maps, T_W, NT, perm = _prep(**inputs)
    nc = _build(T_W, NT)
    trace = bool(int(os.environ.get("KERNEL_TRACE", "0")))
    res = run_bass_kernel_spmd(nc, in_maps, core_ids=list(range(NCORE)), trace=trace)
    LAST_EXEC_NS = res.exec_time_ns
    res_new = np.concatenate([res.results[c]["out"] for c in range(NCORE)], axis=0)
    out = np.empty_like(res_new)
    out[perm] = res_new
    return out



# revision 9
# speedup vs baseline: 1.5060x; 1.5060x over previous
"""ARMA GNN (K=3, T=2, two convs) on 8 TRN2 NeuronCores.

Strategy: nodes sharded 3750/core; edges sharded by dst core, sorted by dst
window (128 nodes); propagate = dma_gather of source rows from a replicated
bf16 table + one-hot scatter matmuls on TensorE into PSUM per dst window.
gcn_norm is folded as propagate(h) = D @ A @ D @ h with D=diag(deg^-1/2):
gather tables are pre-scaled by D (rows), scatter outputs scaled by D
(per-partition multiply). Dense per-stack matmuls run feature-major
([feat, nodes]) and are node-parallel; tables for the next propagate are
built via PE transposes. Cross-core replication of propagate tables uses
ncfw AllGather (bf16).
"""
import os
import numpy as np
import ml_dtypes

import concourse.bass as bass
import concourse.mybir as mybir
import concourse.tile as tile
import concourse.bacc as bacc
from concourse.masks import make_identity
from concourse.bass_utils import run_bass_kernel_spmd

NCORE = 8
N = 30000
EDG = 300000
NL = N // NCORE          # 3750 nodes per core
K = 3
F_IN = 256
F_HID = 256
F_OUT = 64
NWIN = (NL + 127) // 128  # 30 windows (last has 38 nodes)
P = 128
BF = mybir.dt.bfloat16
F32 = mybir.dt.float32
I16 = mybir.dt.int16

LAST_EXEC_NS = None


def _t1_layout():
    if NL < 3 * P:
        return [NL]
    a = ((NL // 3 + P - 1) // P) * P
    return [a, a, NL - 2 * a]


def _wrap16(idx):
    """int16 idx array -> [128, len/16] gather layout (16-part wrap, 8x tile)."""
    w = idx.reshape(-1, 16).T.astype(np.int16).copy()
    return np.tile(w, (8, 1))


def _prep(x, edge_index, init_w1, w1, root_w1, bias1, init_w2, w2, root_w2, bias2):
    """Host-side sharding/staging. Returns (in_maps, T_W, NT)."""
    x = np.asarray(x, np.float32)
    ei = np.asarray(edge_index, np.int64)
    row0, col0 = ei[0], ei[1]
    deg0 = np.bincount(col0, minlength=N)

    # degree-balanced node permutation: serpentine-deal nodes (sorted by
    # in-degree) over the 240 (core, window) bins so per-window edge maxima
    # shrink -> fewer padded scatter tiles (T_W).
    TAIL = NL - (NWIN - 1) * P                      # 38 nodes in last window
    bins = [(c, w) for w in range(NWIN - 1) for c in range(NCORE)]
    tails = [(c, NWIN - 1) for c in range(NCORE)]
    cap = {b: P for b in bins}
    for b in tails:
        cap[b] = TAIL
    order_nodes = np.argsort(-deg0, kind="stable")
    allbins = bins + tails
    assign = {b: [] for b in allbins}
    r = 0
    ptr0 = 0
    while ptr0 < N:
        seq = allbins if (r % 2 == 0) else allbins[::-1]
        for b in seq:
            if ptr0 >= N:
                break
            if len(assign[b]) < cap[b]:
                assign[b].append(order_nodes[ptr0])
                ptr0 += 1
        r += 1
    perm = np.empty(N, np.int64)                    # perm[new] = old
    for (c, w), lst in assign.items():
        base = c * NL + w * P
        perm[base:base + len(lst)] = lst
    inv = np.empty(N, np.int64)
    inv[perm] = np.arange(N)

    row, col = inv[row0], inv[col0]
    x = x[perm]
    deg = np.bincount(col, minlength=N).astype(np.float32)
    dis = np.where(deg > 0, 1.0 / np.sqrt(np.maximum(deg, 1.0)), 0.0).astype(np.float32)

    xs = (x * dis[:, None]).astype(ml_dtypes.bfloat16)          # gather table p0
    x_bf = x.astype(ml_dtypes.bfloat16)

    # per-core edge lists grouped by dst window
    core_of = col // NL
    dloc = col - core_of * NL
    win = dloc // P
    rel = dloc - win * P
    # counts per (core, window)
    cnt = np.zeros((NCORE, NWIN), np.int64)
    np.add.at(cnt, (core_of, win), 1)
    T_W = [int(np.ceil(cnt[:, w].max() / P)) for w in range(NWIN)]
    T_W = [max(t, 1) for t in T_W]
    NT = sum(T_W)
    EP = NT * P

    src_pad = np.zeros((NCORE, EP), np.int64)
    rel_pad = np.full((NCORE, EP), -1, np.int64)
    order = np.lexsort((win, core_of))
    srcs = row[order]
    rels = rel[order]
    cores_s = core_of[order]
    wins_s = win[order]
    # start offsets per (core, window) in the sorted array
    ptr = 0
    woff = np.concatenate([[0], np.cumsum(np.array(T_W) * P)])[:-1]
    for c in range(NCORE):
        for w in range(NWIN):
            n = cnt[c, w]
            if n:
                sl = slice(ptr, ptr + n)
                assert (cores_s[sl] == c).all() and (wins_s[sl] == w).all()
                src_pad[c, woff[w]:woff[w] + n] = srcs[sl]
                rel_pad[c, woff[w]:woff[w] + n] = rels[sl]
                ptr += n
    assert ptr == EDG

    # weights (bf16, host-folded)
    iw1 = np.asarray(init_w1, np.float32)
    w1f = np.asarray(w1, np.float32)
    rw1 = np.asarray(root_w1, np.float32)
    b1 = np.asarray(bias1, np.float32).reshape(K, F_HID)
    iw2 = np.asarray(init_w2, np.float32)
    w2f = np.asarray(w2, np.float32)
    rw2 = np.asarray(root_w2, np.float32)
    b2 = np.asarray(bias2, np.float32).reshape(K, F_OUT)

    def pack_pair(wm):  # [K, 256, 64] -> planes [2, 256, 128]
        pl = np.zeros((2, F_HID, 128), np.float32)
        pl[0, :, :64] = wm[0]
        pl[0, :, 64:] = wm[1]
        pl[1, :, :64] = wm[2]
        return pl

    rw2p = pack_pair(rw2 / 3.0).astype(ml_dtypes.bfloat16)
    iw2p = pack_pair(iw2 / 3.0).astype(ml_dtypes.bfloat16)
    w23 = np.zeros((P, 2, 64), np.float32)   # partition-packed w2/3
    w23[0:64, 0] = w2f[0] / 3.0
    w23[64:128, 0] = w2f[1] / 3.0
    w23[0:64, 1] = w2f[2] / 3.0
    wrm = (rw2.sum(axis=0) / 9.0).astype(ml_dtypes.bfloat16)   # y' = 3y
    b2p = np.zeros((P, 2), np.float32)
    b2p[0:64, 0] = b2[0]
    b2p[64:128, 0] = b2[1]
    b2p[0:64, 1] = b2[2]
    brm = np.zeros((P, 1), np.float32)
    brm[0:64, 0] = b2.sum(axis=0) / 3.0
    b1sb = np.zeros((P, K * 2), np.float32)
    for k in range(K):
        for fo in range(2):
            b1sb[:, k * 2 + fo] = b1[k, fo * P:(fo + 1) * P]

    common = {
        "xs": xs,
        "rw1": rw1.astype(ml_dtypes.bfloat16),
        "iw1": iw1.astype(ml_dtypes.bfloat16),
        "w1": w1f.astype(ml_dtypes.bfloat16),
        "rw2p": rw2p, "iw2p": iw2p,
        "w23": w23.astype(ml_dtypes.bfloat16),
        "wrm": wrm, "b2p": b2p, "brm": brm, "b1": b1sb,
    }
    in_maps = []
    for c in range(NCORE):
        sl = slice(c * NL, (c + 1) * NL)
        disw = np.zeros((P, NWIN), np.float32)
        for w in range(NWIN):
            n = min(P, NL - w * P)
            disw[:n, w] = dis[c * NL + w * P: c * NL + w * P + n]
        m = dict(common)
        m["xT"] = np.ascontiguousarray(x_bf[sl].T)              # [256, 3750]
        # host-built one-hot scatter tiles: oh[p, gt*P+j] = (rel[gt*P+p]==j)
        relm = rel_pad[c].reshape(NT, P)
        ohm = (relm[:, :, None] == np.arange(P)[None, None, :])
        m["oh"] = np.ascontiguousarray(
            ohm.transpose(1, 0, 2).reshape(P, NT * P)).astype(ml_dtypes.bfloat16)
        m["srcidx"] = _wrap16(src_pad[c])
        r1 = src_pad[c] % NL
        c1 = src_pad[c] // NL
        CHS = np.array(_t1_layout())
        cums = np.concatenate([[0], np.cumsum(CHS)])
        j1 = np.searchsorted(cums, r1, side="right") - 1
        m["srcidx1"] = _wrap16(NCORE * cums[j1] + c1 * CHS[j1] + (r1 - cums[j1]))
        m["disw"] = disw
        in_maps.append(m)
    return in_maps, T_W, NT, perm


def _build(T_W, NT):
    NTT = sum(T_W)
    assert NTT == NT
    EP = NT * P
    nc = bacc.Bacc("TRN2", target_bir_lowering=False, debug=False, num_devices=NCORE,
                   num_swdge_queues=4)
    dp = nc.declare_dram_parameter
    xs_t = dp("xs", [N, F_IN], BF, isOutput=False)
    xT_t = dp("xT", [F_IN, NL], BF, isOutput=False)
    src_t = dp("srcidx", [P, EP // 16], I16, isOutput=False)
    src1_t = dp("srcidx1", [P, EP // 16], I16, isOutput=False)
    oh_t = dp("oh", [P, NT * P], BF, isOutput=False)
    disw_t = dp("disw", [P, NWIN], F32, isOutput=False)
    rw1_t = dp("rw1", [K, F_IN, F_HID], BF, isOutput=False)
    iw1_t = dp("iw1", [K, F_HID, F_HID], BF, isOutput=False)
    w1_t = dp("w1", [K, F_HID, F_HID], BF, isOutput=False)
    rw2p_t = dp("rw2p", [2, F_HID, P], BF, isOutput=False)
    iw2p_t = dp("iw2p", [2, F_HID, P], BF, isOutput=False)
    w23_t = dp("w23", [P, 2, 64], BF, isOutput=False)
    wrm_t = dp("wrm", [F_HID, F_OUT], BF, isOutput=False)
    b1_t = dp("b1", [P, K * 2], F32, isOutput=False)
    b2p_t = dp("b2p", [P, 2], F32, isOutput=False)
    brm_t = dp("brm", [P, 1], F32, isOutput=False)
    out_t = dp("out", [NL, F_OUT], F32, isOutput=True)

    CHS = _t1_layout()
    CUMS = np.concatenate([[0], np.cumsum(CHS)]).astype(int)
    t1_locs = [nc.dram_tensor(f"t1_loc{j}", [sz, 768], BF) for j, sz in enumerate(CHS)]
    t2_loc = nc.dram_tensor("t2_loc", [NL, 256], BF)
    t3_loc = nc.dram_tensor("t3_loc", [NL, 128], BF)
    t1_full = nc.dram_tensor("t1_full", [N, 768], BF, addr_space="Shared")
    t2_full = nc.dram_tensor("t2_full", [N, 256], BF, addr_space="Shared")
    t3_full = nc.dram_tensor("t3_full", [N, 128], BF, addr_space="Shared")

    woff_t = np.concatenate([[0], np.cumsum(T_W)])  # tile offsets per window
    # gather chunks of <=8 tiles over the global tile sequence
    chunks = []
    lo = 0
    while lo < NT:
        hi = min(lo + 8, NT)
        chunks.append((lo, hi))
        lo = hi
    chunk_of = {}
    for ci, (lo, hi) in enumerate(chunks):
        for t in range(lo, hi):
            chunk_of[t] = (ci, t - lo)

    NCH = [c for c in range(0, NL, 512)]  # dense node chunk starts

    with tile.TileContext(nc) as tc:
        with (
            tc.tile_pool(name="const", bufs=1) as cp,
            tc.tile_pool(name="resid", bufs=1) as rp,
            tc.tile_pool(name="wk", bufs=4) as wk,
            tc.tile_pool(name="tabp", bufs=5) as tabp,

            tc.tile_pool(name="psT", bufs=2, space="PSUM") as psT,
            tc.tile_pool(name="psD", bufs=2, space="PSUM") as psD,
        ):
            # ---- constants ----
            ident = cp.tile([P, P], BF)
            make_identity(nc, ident[:])
            src16 = cp.tile([P, EP // 16], I16)
            nc.sync.dma_start(out=src16[:], in_=src_t[:])
            src16b = cp.tile([P, EP // 16], I16)
            nc.sync.dma_start(out=src16b[:], in_=src1_t[:])
            disw = cp.tile([P, NWIN], F32)
            nc.sync.dma_start(out=disw[:], in_=disw_t[:])
            b1c = cp.tile([P, K * 2], F32)
            nc.sync.dma_start(out=b1c[:], in_=b1_t[:])
            b2c = cp.tile([P, 2], F32)
            nc.sync.dma_start(out=b2c[:], in_=b2p_t[:])
            brmc = cp.tile([P, 1], F32)
            nc.sync.dma_start(out=brmc[:], in_=brm_t[:])
            # weights: [128, k*2+fi, fout-cols]
            rw1s = cp.tile([P, K * 2, F_HID], BF)
            iw1s = cp.tile([P, K * 2, F_HID], BF)
            w1s = cp.tile([P, K * 2, F_HID], BF)
            for k in range(K):
                for fi in range(2):
                    nc.sync.dma_start(out=rw1s[:, k * 2 + fi, :], in_=rw1_t[k, fi * P:(fi + 1) * P, :])
                    nc.sync.dma_start(out=iw1s[:, k * 2 + fi, :], in_=iw1_t[k, fi * P:(fi + 1) * P, :])
                    nc.sync.dma_start(out=w1s[:, k * 2 + fi, :], in_=w1_t[k, fi * P:(fi + 1) * P, :])
            rw2s = cp.tile([P, 2 * 2, P], BF)
            iw2s = cp.tile([P, 2 * 2, P], BF)
            for pl in range(2):
                for fi in range(2):
                    nc.sync.dma_start(out=rw2s[:, pl * 2 + fi, :], in_=rw2p_t[pl, fi * P:(fi + 1) * P, :])
                    nc.sync.dma_start(out=iw2s[:, pl * 2 + fi, :], in_=iw2p_t[pl, fi * P:(fi + 1) * P, :])
            w23s = cp.tile([P, 2, 64], BF)
            nc.sync.dma_start(out=w23s[:], in_=w23_t[:])
            wrms = cp.tile([P, 2, F_OUT], BF)
            nc.sync.dma_start(out=wrms[:], in_=wrm_t[:].rearrange("(a p) f -> p a f", p=P))
            xT = rp.tile([P, 2, NL], BF, tag="xTt")
            nc.sync.dma_start(out=xT[:], in_=xT_t[:].rearrange("(a p) n -> p a n", p=P))

            # ---- residents ----
            CW = [min(512, NL - b) for b in range(0, NL, 512)]
            root1T = [rp.tile([P, K * 2, CW[j]], BF, tag=f"slC{j}", name=f"root1T{j}") for j in range(len(CW))]
            SxT = [rp.tile([P, 2, CW[j]], BF, tag=f"slB{j}", name=f"SxT{j}") for j in range(len(CW))]
            ypT = [rp.tile([P, 2, CW[j]], BF, tag=f"slA{j}", name=f"ypT{j}") for j in range(len(CW))]
            root2T = [rp.tile([P, 2, CW[j]], BF, tag=f"slB{j}", name=f"root2T{j}") for j in range(len(CW))]
            h3T = [rp.tile([P, 2, CW[j]], BF, tag=f"slC{j}", name=f"h3T{j}") for j in range(len(CW))]
            rmnm = rp.tile([P, NWIN, F_OUT], BF)  # mean_k root2, node-major

            GO = int(os.environ.get("GO", "99"))
            # ---- stage B: root1T = x@rw1 + b1 (feature-major) ----
            if GO >= 1:
                for k in range(K):
                    for fo in range(2):
                        for nb in NCH:
                            nn_ = min(512, NL - nb)
                            ps = psD.tile([P, 512], F32, tag="dense")
                            for fi in range(2):
                                nc.tensor.matmul(
                                    out=ps[:, :nn_],
                                    lhsT=rw1s[:, k * 2 + fi, fo * P:(fo + 1) * P],
                                    rhs=xT[:, fi, nb:nb + nn_],
                                    start=(fi == 0), stop=(fi == 1))
                            nc.vector.tensor_scalar(
                                out=root1T[nb // 512][:, k * 2 + fo, :nn_], in0=ps[:, :nn_],
                                scalar1=b1c[:, k * 2 + fo:k * 2 + fo + 1], scalar2=None,
                                op0=mybir.AluOpType.add)

            # ---- generic propagate ----
            NPROP = int(os.environ.get("NPROP", "99"))
            NAG = int(os.environ.get("NAG", "99"))
            prop_ctr = [0]

            def propagate(table, W, epilogue, idxs=None):
                """scatter-accumulate per window; epilogue(w, psum_acc)."""
                my_idx = prop_ctr[0]
                prop_ctr[0] += 1
                pa = tc.tile_pool(name=f"psA_{table.name}", bufs=(2 if W > 256 else 4), space="PSUM")
                psAx = pa.__enter__()
                if my_idx >= NPROP:
                    for w in range(NWIN):
                        acc = psAx.tile([P, W if W > 256 else 256], F32, tag="acc", name=f"acc_{my_idx}_{w}")
                        nc.vector.memset(acc[:], 0)
                        epilogue(w, acc)
                    pa.__exit__(None, None, None)
                    return
                mp = tc.tile_pool(name=f"msg_{table.name}", bufs=(3 if W > 256 else 5))
                mpool = mp.__enter__()
                op_ = tc.tile_pool(name=f"ohs_{table.name}", bufs=3)
                opool = op_.__enter__()
                msgs = {}
                ohts = {}
                for ci, (lo, hi) in enumerate(chunks):
                    m = mpool.tile([P, 8, W], BF, tag="msg", name=f"msg_{table.name}_{ci}")
                    ni = (hi - lo) * P
                    nc.gpsimd.dma_gather(
                        out_ap=m[:, :hi - lo, :], in_ap=table[:],
                        idxs_ap=(idxs if idxs is not None else src16)[:, lo * 8:lo * 8 + ni // 16],
                        num_idxs=ni, num_idxs_reg=ni, elem_size=W,
                        queue_num=ci % 4)
                    msgs[ci] = m
                    oht = opool.tile([P, 8 * P], BF, tag="ohs", name=f"ohs_{table.name}_{ci}")
                    nc.sync.dma_start(out=oht[:, :(hi - lo) * P], in_=oh_t[:, lo * P:hi * P])
                    ohts[ci] = oht
                for w in range(NWIN):
                    acc = psAx.tile([P, W if W > 256 else 256], F32, tag="acc", name=f"accr_{my_idx}_{w}")
                    for ti in range(T_W[w]):
                        gt = int(woff_t[w]) + ti
                        ci, slot = chunk_of[gt]
                        oh = ohts[ci][:, slot * P:(slot + 1) * P]
                        st = (ti == 0)
                        sp = (ti == T_W[w] - 1)
                        if W > 512:
                            for c0, c1 in ((0, 512), (512, W)):
                                nc.tensor.matmul(out=acc[:, c0:c1], lhsT=oh,
                                                 rhs=msgs[ci][:, slot, c0:c1],
                                                 start=st, stop=sp)
                        else:
                            nc.tensor.matmul(out=acc[:, :W], lhsT=oh,
                                             rhs=msgs[ci][:, slot, :],
                                             start=st, stop=sp)
                    epilogue(w, acc)
                op_.__exit__(None, None, None)
                mp.__exit__(None, None, None)
                pa.__exit__(None, None, None)

            # ---- propagate 0: Sx from xs ----
            if GO >= 2:
                def ep0(w, acc):
                    nn_ = min(P, NL - w * P)
                    snm = wk.tile([P, F_IN], BF, tag="snm")
                    nc.vector.tensor_scalar(out=snm[:], in0=acc[:, :F_IN],
                                            scalar1=disw[:, w:w + 1], scalar2=None,
                                            op0=mybir.AluOpType.mult)
                    for fc in range(2):
                        pt = psT.tile([P, P], BF, tag="tr")
                        nc.tensor.transpose(out=pt[:], in_=snm[:, fc * P:(fc + 1) * P], identity=ident[:])
                        nc.vector.tensor_copy(out=SxT[(w * P) // 512][:, fc, (w * P) % 512:(w * P) % 512 + nn_], in_=pt[:, :nn_])
                propagate(xs_t, F_IN, ep0)

            # ---- stage D: h1 -> g1 -> t1_loc ----
            if GO >= 3:
                for nb in NCH:
                    nn_ = min(512, NL - nb)
                    nsub = [s for s in range(0, nn_, P)]
                    tabs = [tabp.tile([P, 768], BF, tag="tab1", name=f"tab1_{nb}_{i}") for i, _ in enumerate(nsub)]
                    for k in range(K):
                        h1 = [wk.tile([P, 512], BF, tag="h1", name=f"h1_{nb}_{k}_{i}") for i in range(2)]
                        for fo in range(2):
                            ps = psD.tile([P, 512], F32, tag="dense")
                            for fi in range(2):
                                nc.tensor.matmul(out=ps[:, :nn_],
                                                 lhsT=iw1s[:, k * 2 + fi, fo * P:(fo + 1) * P],
                                                 rhs=SxT[nb // 512][:, fi, :nn_],
                                                 start=(fi == 0), stop=False)
                            nc.tensor.matmul(out=ps[:, :nn_], lhsT=ident[:],
                                             rhs=root1T[nb // 512][:, k * 2 + fo, :nn_],
                                             start=False, stop=True)
                            nc.scalar.activation(out=h1[fo][:, :nn_], in_=ps[:, :nn_],
                                                 func=mybir.ActivationFunctionType.Relu)
                        for fo in range(2):
                            ps = psD.tile([P, 512], F32, tag="dense")
                            for fi in range(2):
                                nc.tensor.matmul(out=ps[:, :nn_],
                                                 lhsT=w1s[:, k * 2 + fi, fo * P:(fo + 1) * P],
                                                 rhs=h1[fi][:, :nn_],
                                                 start=(fi == 0), stop=(fi == 1))
                            gb = wk.tile([P, 512], BF, tag="gb")
                            nc.vector.tensor_copy(out=gb[:, :nn_], in_=ps[:, :nn_])
                            for si, s in enumerate(nsub):
                                pt = psT.tile([P, P], BF, tag="tr")
                                nc.tensor.transpose(out=pt[:], in_=gb[:, s:s + P], identity=ident[:])
                                w0 = (nb + s) // P
                                nc.vector.tensor_scalar(
                                    out=tabs[si][:, k * 256 + fo * P:k * 256 + (fo + 1) * P],
                                    in0=pt[:], scalar1=disw[:, w0:w0 + 1], scalar2=None,
                                    op0=mybir.AluOpType.mult)
                    for si, s in enumerate(nsub):
                        r = nb + s
                        rows = min(P, NL - r)
                        j = int(np.searchsorted(CUMS, r, side="right")) - 1
                        nc.sync.dma_start(out=t1_locs[j][r - CUMS[j]:r - CUMS[j] + rows, :],
                                          in_=tabs[si][:rows, :])

                bnds = [(NCORE * CUMS[j], NCORE * CUMS[j + 1]) for j in range(len(CHS))]
                if NAG > prop_ctr[0] - 1:
                    for j in range(len(CHS)):
                        nc.gpsimd.collective_compute(
                            "AllGather", mybir.AluOpType.bypass,
                            replica_groups=[list(range(NCORE))],
                            ins=[t1_locs[j][:]], outs=[t1_full[bnds[j][0]:bnds[j][1], :]])
                else:
                    for j in range(len(CHS)):
                        nc.sync.dma_start(out=t1_full[bnds[j][0]:bnds[j][0] + CHS[j], :], in_=t1_locs[j][:])

            # ---- propagate 1 + h2 + y' ----
            if GO >= 4:
                def ep1(w, acc):
                    nn_ = min(P, NL - w * P)
                    snm = wk.tile([P, 768], BF, tag="snm1")
                    nc.vector.tensor_scalar(out=snm[:], in0=acc[:],
                                            scalar1=disw[:, w:w + 1], scalar2=None,
                                            op0=mybir.AluOpType.mult)
                    for k in range(K):
                        for fo in range(2):
                            pt = psT.tile([P, P], BF, tag="tr")
                            nc.tensor.transpose(out=pt[:], in_=snm[:, k * 256 + fo * P:k * 256 + (fo + 1) * P],
                                                identity=ident[:])
                            hsum = wk.tile([P, P], BF, tag="hsum")
                            nc.vector.tensor_tensor(out=hsum[:, :nn_], in0=pt[:, :nn_],
                                                    in1=root1T[(w * P) // 512][:, k * 2 + fo, (w * P) % 512:(w * P) % 512 + nn_],
                                                    op=mybir.AluOpType.add)
                            if k == 0:
                                nc.scalar.activation(out=ypT[(w * P) // 512][:, fo, (w * P) % 512:(w * P) % 512 + nn_], in_=hsum[:, :nn_],
                                                     func=mybir.ActivationFunctionType.Relu)
                            else:
                                hr = wk.tile([P, P], BF, tag="hr")
                                nc.scalar.activation(out=hr[:, :nn_], in_=hsum[:, :nn_],
                                                     func=mybir.ActivationFunctionType.Relu)
                                nc.vector.tensor_tensor(out=ypT[(w * P) // 512][:, fo, (w * P) % 512:(w * P) % 512 + nn_],
                                                        in0=ypT[(w * P) // 512][:, fo, (w * P) % 512:(w * P) % 512 + nn_],
                                                        in1=hr[:, :nn_], op=mybir.AluOpType.add)
                propagate(t1_full, 768, ep1, idxs=src16b)

            # ---- stage G: root2T, rm, g2 -> t2_loc ----
            if GO >= 5:
                for nb in NCH:
                    nn_ = min(512, NL - nb)
                    nsub = [s for s in range(0, nn_, P)]
                    tabs = [tabp.tile([P, 256], BF, tag="tab2", name=f"tab2_{nb}_{i}") for i, _ in enumerate(nsub)]
                    for pl in range(2):
                        # root2T plane
                        ps = psD.tile([P, 512], F32, tag="dense")
                        for fi in range(2):
                            nc.tensor.matmul(out=ps[:, :nn_], lhsT=rw2s[:, pl * 2 + fi, :],
                                             rhs=ypT[nb // 512][:, fi, :nn_],
                                             start=(fi == 0), stop=(fi == 1))
                        nc.vector.tensor_scalar(out=root2T[nb // 512][:, pl, :nn_], in0=ps[:, :nn_],
                                                scalar1=b2c[:, pl:pl + 1], scalar2=None,
                                                op0=mybir.AluOpType.add)
                        # g2 plane
                        ps2 = psD.tile([P, 512], F32, tag="dense")
                        for fi in range(2):
                            nc.tensor.matmul(out=ps2[:, :nn_], lhsT=iw2s[:, pl * 2 + fi, :],
                                             rhs=ypT[nb // 512][:, fi, :nn_],
                                             start=(fi == 0), stop=(fi == 1))
                        gb = wk.tile([P, 512], BF, tag="gb")
                        nc.vector.tensor_copy(out=gb[:, :nn_], in_=ps2[:, :nn_])
                        for si, s in enumerate(nsub):
                            pt = psT.tile([P, P], BF, tag="tr")
                            nc.tensor.transpose(out=pt[:], in_=gb[:, s:s + P], identity=ident[:])
                            w0 = (nb + s) // P
                            if pl == 0:
                                nc.vector.tensor_scalar(out=tabs[si][:, 0:P], in0=pt[:],
                                                        scalar1=disw[:, w0:w0 + 1], scalar2=None,
                                                        op0=mybir.AluOpType.mult)
                            else:
                                nc.vector.tensor_scalar(out=tabs[si][:, P:P + 64], in0=pt[:, :64],
                                                        scalar1=disw[:, w0:w0 + 1], scalar2=None,
                                                        op0=mybir.AluOpType.mult)
                                nc.vector.memset(tabs[si][:, 192:256], 0)
                    # rm (mean_k root2) node-major
                    psr = psD.tile([P, 512], F32, tag="dense")
                    for fi in range(2):
                        nc.tensor.matmul(out=psr[:64, :nn_], lhsT=wrms[:, fi, :],
                                         rhs=ypT[nb // 512][:, fi, :nn_],
                                         start=(fi == 0), stop=(fi == 1))
                    rmf = wk.tile([P, 512], BF, tag="rmf")
                    nc.vector.tensor_scalar(out=rmf[:64, :nn_], in0=psr[:64, :nn_],
                                            scalar1=brmc[:64, 0:1], scalar2=None,
                                            op0=mybir.AluOpType.add)
                    for si, s in enumerate(nsub):
                        pt = psT.tile([P, P], BF, tag="tr")
                        nc.tensor.matmul(out=pt[:, :64], lhsT=rmf[:64, s:s + P],
                                         rhs=ident[:64, :64], is_transpose=True)
                        w0 = (nb + s) // P
                        nc.vector.tensor_copy(out=rmnm[:, w0, :], in_=pt[:, :64])
                    for si, s in enumerate(nsub):
                        rows = min(P, NL - (nb + s))
                        nc.sync.dma_start(out=t2_loc[nb + s:nb + s + rows, :], in_=tabs[si][:rows, :])

                if NAG > prop_ctr[0] - 1:
                    nc.gpsimd.collective_compute(
                        "AllGather", mybir.AluOpType.bypass,
                        replica_groups=[list(range(NCORE))],
                        ins=[t2_loc[:]], outs=[t2_full[:]])
                else:
                    nc.sync.dma_start(out=t2_full[0:NL, :], in_=t2_loc[:])

            # ---- propagate 2 + h3 ----
            if GO >= 6:
                def ep2(w, acc):
                    nn_ = min(P, NL - w * P)
                    snm = wk.tile([P, 256], BF, tag="snm2")
                    nc.vector.tensor_scalar(out=snm[:], in0=acc[:, :256],
                                            scalar1=disw[:, w:w + 1], scalar2=None,
                                            op0=mybir.AluOpType.mult)
                    for pl in range(2):
                        pt = psT.tile([P, P], BF, tag="tr")
                        nc.tensor.transpose(out=pt[:], in_=snm[:, pl * P:(pl + 1) * P], identity=ident[:])
                        nc.vector.tensor_tensor(out=h3T[(w * P) // 512][:, pl, (w * P) % 512:(w * P) % 512 + nn_], in0=pt[:, :nn_],
                                                in1=root2T[(w * P) // 512][:, pl, (w * P) % 512:(w * P) % 512 + nn_],
                                                op=mybir.AluOpType.add)
                propagate(t2_full, 256, ep2)

            # ---- stage: gm = mean_k g3 -> t3_loc ----
            if GO >= 7:
                for nb in NCH:
                    nn_ = min(512, NL - nb)
                    nsub = [s for s in range(0, nn_, P)]
                    ps = psD.tile([P, 512], F32, tag="dense")
                    nc.tensor.matmul(out=ps[:64, :nn_], lhsT=w23s[:, 0, :],
                                     rhs=h3T[nb // 512][:, 0, :nn_],
                                     start=True, stop=False)
                    nc.tensor.matmul(out=ps[:64, :nn_], lhsT=w23s[0:64, 1, :],
                                     rhs=h3T[nb // 512][0:64, 1, :nn_],
                                     start=False, stop=True)
                    gmf = wk.tile([P, 512], BF, tag="gmf")
                    nc.vector.tensor_copy(out=gmf[:64, :nn_], in_=ps[:64, :nn_])
                    for si, s in enumerate(nsub):
                        pt = psT.tile([P, P], BF, tag="tr")
                        nc.tensor.matmul(out=pt[:, :64], lhsT=gmf[:64, s:s + P],
                                         rhs=ident[:64, :64], is_transpose=True)
                        w0 = (nb + s) // P
                        tb = tabp.tile([P, 128], BF, tag="tab3")
                        nc.vector.tensor_scalar(out=tb[:, :64], in0=pt[:, :64],
                                                scalar1=disw[:, w0:w0 + 1], scalar2=None,
                                                op0=mybir.AluOpType.mult)
                        nc.vector.memset(tb[:, 64:], 0)
                        rows = min(P, NL - (nb + s))
                        nc.sync.dma_start(out=t3_loc[nb + s:nb + s + rows, :], in_=tb[:rows, :])

                if NAG > prop_ctr[0] - 1:
                    nc.gpsimd.collective_compute(
                        "AllGather", mybir.AluOpType.bypass,
                        replica_groups=[list(range(NCORE))],
                        ins=[t3_loc[:]], outs=[t3_full[:]])
                else:
                    nc.sync.dma_start(out=t3_full[0:NL, :], in_=t3_loc[:])

            # ---- propagate 3 + logsoftmax -> out ----
            if GO >= 8:
                def ep3(w, acc):
                    rows = min(P, NL - w * P)
                    z = wk.tile([P, 64], F32, tag="z")
                    nc.vector.tensor_scalar(out=z[:], in0=acc[:, :64],
                                            scalar1=disw[:, w:w + 1], scalar2=None,
                                            op0=mybir.AluOpType.mult)
                    nc.vector.tensor_tensor(out=z[:], in0=z[:], in1=rmnm[:, w, :],
                                            op=mybir.AluOpType.add)
                    if int(os.environ.get("SKIP_SOFTMAX", "0")):
                        nc.sync.dma_start(out=out_t[w * P:w * P + rows, :], in_=z[:rows, :])
                        return
                    mx = wk.tile([P, 1], F32, tag="mx")
                    nc.vector.tensor_reduce(out=mx[:], in_=z[:], axis=mybir.AxisListType.X,
                                            op=mybir.AluOpType.max)
                    nmx = wk.tile([P, 1], F32, tag="nmx")
                    nc.vector.tensor_scalar(out=nmx[:], in0=mx[:], scalar1=-1.0, scalar2=None,
                                            op0=mybir.AluOpType.mult)
                    ex = wk.tile([P, 64], F32, tag="ex")
                    sume = wk.tile([P, 1], F32, tag="sume")
                    nc.scalar.activation(out=ex[:], in_=z[:],
                                         func=mybir.ActivationFunctionType.Exp,
                                         bias=nmx[:, 0:1], scale=1.0, accum_out=sume[:, 0:1])
                    lse = wk.tile([P, 1], F32, tag="lse")
                    nc.scalar.activation(out=lse[:], in_=sume[:],
                                         func=mybir.ActivationFunctionType.Ln)
                    o = wk.tile([P, 64], F32, tag="o")
                    nc.vector.tensor_scalar(out=o[:], in0=z[:], scalar1=mx[:, 0:1],
                                            scalar2=lse[:, 0:1],
                                            op0=mybir.AluOpType.subtract,
                                            op1=mybir.AluOpType.subtract)
                    nc.sync.dma_start(out=out_t[w * P:w * P + rows, :], in_=o[:rows, :])
                propagate(t3_full, 128, ep3)

    nc.compile()
    return nc


def kernel(**inputs):
    global LAST_EXEC_NS
    in_maps, T_W, NT, perm = _prep(**inputs)
    nc = _build(T_W, NT)
    trace = bool(int(os.environ.get("KERNEL_TRACE", "0")))
    res = run_bass_kernel_spmd(nc, in_maps, core_ids=list(range(NCORE)), trace=trace)
    LAST_EXEC_NS = res.exec_time_ns
    res_new = np.concatenate([res.results[c]["out"] for c in range(NCORE)], axis=0)
    out = np.empty_like(res_new)
    out[perm] = res_new
    return out

